# revision 29
# baseline (speedup 1.0000x reference)
"""Trainium2 Bass kernel for nn_DTN_47459388620856 (grouped-moment2 norm +
2x2 pooled positional-attention renormalization).

v5 — bf16 end-to-end, engine rebalance, software pipelining (HW-probed):
  * x cast to bf16 on host; output returned bf16, upcast on host. Halves
    DMA traffic and makes big DVE ops eligible for the 2-byte 2x mode
    (probed: all-bf16 unit-stride tensor_tensor = 2x; broadcast/mixed/
    PSUM = 1x; tensor_scalar with fp32 scalar-AP = 2x; reduce = 1x).
  * DVE (2x): xn via 16 per-(q,h) tensor_scalar_mul, halve-halve+reduce
    for m2/sxn, pooling, per-q num/out, V0 (1x, PSUM read).
  * ACT: xsq Square (split in halves to overlap the DVE m2 chain),
    S rsqrt, xpq/sqa squares, per-q Abq via Identity+bias-AP,
    per-q iv via raw Rsqrt+bias-AP.
  * GpSimd only issues store DMAs: bulk GpSimd work both runs slow
    (~0.5 elem/lane/cycle) and, measured, contends SBUF with DVE,
    slowing DVE 2x ops 2-3x. Keep it idle.
  * sB^2 folded into posB stationary (xpq is a plain square).
  * Emission is software-pipelined: iteration `it` interleaves batch
    `it`'s front half (loads, xsq/m2, S, xn/pool, A-matmuls, xpq,
    B-matmuls) with batch `it-1`'s back half (sqa/V0, Abq, iv, num/out,
    stores) so the in-order DVE/ACT streams rarely stall.
"""

import numpy as np


def _ensure_path():
    try:
        import concourse  # noqa: F401
    except ImportError:
        import sys
        for p in ("/opt/trn_rl_repo",):
            if p not in sys.path:
                sys.path.insert(0, p)


EPS = 1e-5
HEADS, RES, PS = 4, 28, 14
T, C = RES * RES, 768
CH = C // HEADS
P = PS * PS
JT = 98
NCORES = 8
BP = 4

_PROGRAM_CACHE = {}


def _sigmoid(v):
    return 1.0 / (1.0 + np.exp(-v.astype(np.float64)))


def _host_consts(mean_norm_weight, var_norm_weight, pos_w, pos_b):
    import ml_dtypes
    mw = _sigmoid(mean_norm_weight)
    vw = _sigmoid(var_norm_weight)

    ind = np.arange(PS)[None, :] - np.arange(PS)[:, None]
    indx = np.tile(ind, (PS, PS))
    indy = np.repeat(np.repeat(ind, PS, axis=0), PS, axis=1)
    rel = np.stack([indx, indy, indx * indx + indy * indy], -1).astype(np.float32)
    scores = rel @ pos_w.T.astype(np.float32) + pos_b.astype(np.float32)
    e = np.exp(scores - scores.max(axis=0, keepdims=True))
    pos = e / e.sum(axis=0, keepdims=True)
    pos_h = np.transpose(pos, (2, 0, 1)).astype(np.float64)   # (H, i, j)

    # posA folds (1-mw)/4 so A = posA @ xp_sum = (1-mw)*mean_r;
    # posB folds sB^2 so B = posB @ xp_sum^2 = (1-vw)*mean2_r.
    sA = ((1.0 - mw) / 4.0)
    sB2 = ((1.0 - vw) / 16.0)
    posA = np.zeros((2, JT, HEADS, 2, JT), np.float32)
    posB = np.zeros((2, JT, HEADS, 2, JT), np.float32)
    for ic in range(2):
        for jc in range(2):
            blk = np.transpose(
                pos_h[:, ic * JT:(ic + 1) * JT, jc * JT:(jc + 1) * JT],
                (1, 0, 2))
            posB[ic, :, :, jc, :] = blk * sB2[None, :, None]
            posA[ic, :, :, jc, :] = blk * sA[None, :, None]
    posA_bf = posA.astype(ml_dtypes.bfloat16)
    posB_bf = posB.astype(ml_dtypes.bfloat16)

    sB = (np.sqrt(1.0 - vw) / 4.0).astype(np.float32)
    sG = (np.sqrt(1.0 - vw) / (1.0 - mw)).astype(np.float32)
    return posA_bf, posB_bf, sB, sG, mw.astype(np.float32), vw.astype(np.float32)


def _raw_act(eng, out, in_, func, mybir, bias=0.0, scale=1.0):
    ins = [eng.lower_ap(in_)]
    ins.append(eng.lower_ap(bias) if not isinstance(bias, float)
               else mybir.ImmediateValue(dtype=mybir.dt.float32, value=bias))
    ins.append(mybir.ImmediateValue(dtype=mybir.dt.float32, value=scale))
    ins.append(mybir.ImmediateValue(dtype=mybir.dt.float32, value=0.0))
    return eng.add_instruction(
        mybir.InstActivation(
            name=eng.bass.get_next_instruction_name(),
            func=func, ins=ins, outs=[eng.lower_ap(out)]))


def _build_program(consts):
    _ensure_path()
    from contextlib import ExitStack
    import concourse.bass as bass  # noqa: F401
    import concourse.tile as tile
    from concourse import bacc, mybir

    posA_bf, posB_bf, sB, sG, mw, vw = consts
    eqh = bool(np.all(mw == mw[0]) and np.all(vw == vw[0]))
    assert eqh, "v5 kernel assumes per-head norm weights are equal"

    dt = mybir.dt.float32
    bt = mybir.dt.bfloat16
    AO = mybir.AluOpType
    AF = mybir.ActivationFunctionType
    AX = mybir.AxisListType

    nc = bacc.Bacc("TRN2", target_bir_lowering=False, debug=False,
                   enable_asserts=False)

    x_d = nc.dram_tensor("x", (BP, 14, 2, 14, 2, C), bt,
                         kind="ExternalInput").ap()
    pA_d = nc.dram_tensor("posA", (2, JT, HEADS, 2, JT), bt,
                          kind="ExternalInput").ap()
    pB_d = nc.dram_tensor("posB", (2, JT, HEADS, 2, JT), bt,
                          kind="ExternalInput").ap()
    out_d = nc.dram_tensor("out", (BP, 14, 2, 14, 2, C), bt,
                           kind="ExternalOutput").ap()

    x_re = x_d.transpose([0, 1, 3, 2, 4, 5])
    o_re = out_d.transpose([0, 1, 3, 2, 4, 5])

    with ExitStack() as ctx:
        tc = ctx.enter_context(tile.TileContext(nc))
        cpool = ctx.enter_context(tc.tile_pool(name="consts", bufs=1))
        xtp = ctx.enter_context(tc.tile_pool(name="xt", bufs=3))
        xnp = ctx.enter_context(tc.tile_pool(name="xn", bufs=2))
        xqp = ctx.enter_context(tc.tile_pool(name="xsq", bufs=2))
        smp = ctx.enter_context(tc.tile_pool(name="smalls", bufs=2))
        plp = ctx.enter_context(tc.tile_pool(name="pool", bufs=2))
        abp = ctx.enter_context(tc.tile_pool(name="ab", bufs=2))
        nmp = ctx.enter_context(tc.tile_pool(name="num", bufs=2))
        obp = ctx.enter_context(tc.tile_pool(name="outsb", bufs=2))
        ppp = ctx.enter_context(tc.tile_pool(name="ppsum", bufs=2,
                                             space="PSUM"))

        xt0s = []
        for jc in range(2):
            xt = xtp.tile([JT, 2, 2, C], bt, tag="xt")
            for d in range(2):
                nc.sync.dma_start(xt[:, d],
                                  x_re[0, jc * 7:(jc + 1) * 7, :, d])
            xt0s.append(xt)
        pA_sb, pB_sb = [], []
        for ic in range(2):
            tA = cpool.tile([JT, HEADS, 2, JT], bt, tag=f"posA{ic}")
            nc.sync.dma_start(tA[:], pA_d[ic])
            pA_sb.append(tA)
            tB = cpool.tile([JT, HEADS, 2, JT], bt, tag=f"posB{ic}")
            nc.sync.dma_start(tB[:], pB_d[ic])
            pB_sb.append(tB)

        # Software-pipelined: iteration `it` emits batch `it`'s front half
        # (loads, xsq/m2, S, xn/pool, matmuls) interleaved with batch
        # `it-1`'s back half (sqa/V0, Abq, iv, num/out, stores), so neither
        # the in-order DVE nor the in-order ACT stream ever stalls long.
        prev = None
        for it in range(BP + 1):
            b = it
            # -- prev back-half part 1: sqa (ACT, reads PSUM A-region) + V0
            if prev is not None:
                for jc in range(2):
                    pt = prev["pts"][jc]
                    sqa = abp.tile([JT, HEADS, CH], bt, tag="sqa")
                    nc.scalar.activation(sqa[:], pt[:, :, 0:CH], AF.Square,
                                         scale=float(sG[0]))
                    V0 = abp.tile([JT, HEADS, CH], bt, tag="V0")
                    nc.vector.tensor_sub(V0[:], pt[:, :, CH:2 * CH], sqa[:])
                    prev[f"V0{jc}"] = V0

            cur = None
            if b < BP:
                cur = {"xts": [], "m2s": [], "Ss": [], "xns": [],
                       "xpss": [], "xpqs": [], "mmls": [], "svls": [],
                       "pts": [], "b": b}
                # front 1: loads + xsq + halve + m2 (DVE 2x)
                for jc in range(2):
                    if b == 0:
                        xt = xt0s[jc]
                    else:
                        xt = xtp.tile([JT, 2, 2, C], bt, tag="xt")
                        for d in range(2):
                            nc.sync.dma_start(
                                xt[:, d],
                                x_re[b, jc * 7:(jc + 1) * 7, :, d])
                    cur["xts"].append(xt)
                    xseg = xt[:].rearrange("p d s (h c) -> p (d s h) c",
                                           h=HEADS)
                    xsq = xqp.tile([JT, 16, CH], bt, tag="xsq")
                    m2h = xqp.tile([JT, 16, CH // 2], bt, tag="m2h")
                    m2q = xqp.tile([JT, 16, CH // 4], bt, tag="m2q")
                    m2 = smp.tile([JT, 16], dt, tag="m2")
                    for hf in range(2):
                        sl = slice(hf * 8, (hf + 1) * 8)
                        nc.scalar.activation(xsq[:, sl], xseg[:, sl],
                                             AF.Square)
                        nc.vector.tensor_add(m2h[:, sl],
                                             xsq[:, sl, 0:CH // 2],
                                             xsq[:, sl, CH // 2:CH])
                        nc.vector.tensor_add(m2q[:, sl],
                                             m2h[:, sl, 0:CH // 4],
                                             m2h[:, sl, CH // 4:CH // 2])
                        nc.vector.reduce_sum(m2[:, sl], m2q[:, sl],
                                             axis=AX.X)
                    cur["m2s"].append(m2)
                # front 2: S (ACT Rsqrt)
                for jc in range(2):
                    S = smp.tile([JT, 16], dt, tag="S")
                    _raw_act(nc.scalar, S[:], cur["m2s"][jc][:], AF.Rsqrt,
                             mybir, bias=EPS, scale=1.0 / CH)
                    cur["Ss"].append(S)

            # -- prev back-half part 2: Abq (ACT Identity x8)
            if prev is not None:
                for jc in range(2):
                    pt = prev["pts"][jc]
                    Abq = nmp.tile([JT, 4, C], bt, tag="Abq")
                    for q in range(4):
                        nc.scalar.activation(
                            Abq[:, q, :].rearrange(
                                "p (h c) -> p h c", h=HEADS),
                            pt[:, :, 0:CH], AF.Identity,
                            bias=prev["mmls"][jc][:, q:q + 1])
                    prev[f"Abq{jc}"] = Abq

            if cur is not None:
                # front 3: xn (DVE ts_mul 2x) + sxn + smalls + pooling
                for jc in range(2):
                    xt, m2, S = (cur["xts"][jc], cur["m2s"][jc],
                                 cur["Ss"][jc])
                    xn = xnp.tile([JT, 4, C], bt, tag="xn")
                    for q in range(4):
                        d, s_ = divmod(q, 2)
                        for h in range(HEADS):
                            nc.vector.tensor_scalar_mul(
                                xn[:, q, h * CH:(h + 1) * CH],
                                xt[:, d, s_, h * CH:(h + 1) * CH],
                                S[:, q * HEADS + h:q * HEADS + h + 1])
                    cur["xns"].append(xn)

                    tmp2 = plp.tile([JT, 2, C], bt, tag="tmp2")
                    nc.vector.tensor_add(tmp2[:], xn[:, 0:2, :],
                                         xn[:, 2:4, :])
                    xps = plp.tile([JT, C], bt, tag="xps")
                    nc.vector.tensor_add(xps[:], tmp2[:, 0, :],
                                         tmp2[:, 1, :])
                    cur["xpss"].append(xps)
                    xpq = plp.tile([JT, C], bt, tag="xpq")
                    nc.scalar.activation(xpq[:], xps[:], AF.Square)
                    cur["xpqs"].append(xpq)

                    xnh = xnp.tile([JT, 4, C // 2], bt, tag="xnh")
                    nc.vector.tensor_add(xnh[:], xn[:, :, 0:C // 2],
                                         xn[:, :, C // 2:C])
                    xnq = xnp.tile([JT, 4, C // 4], bt, tag="xnq")
                    nc.vector.tensor_add(xnq[:], xnh[:, :, 0:C // 4],
                                         xnh[:, :, C // 4:C // 2])
                    sxn = smp.tile([JT, 4], dt, tag="sxn")
                    nc.vector.reduce_sum(sxn[:], xnq[:], axis=AX.X)

                    r_ = smp.tile([JT, 16], dt, tag="r")
                    nc.any.tensor_mul(r_[:], S[:], S[:])
                    u = smp.tile([JT, 16], dt, tag="u")
                    nc.any.tensor_mul(u[:], m2[:], r_[:])
                    su = smp.tile([JT, 4], dt, tag="su")
                    nc.vector.reduce_sum(
                        su[:], u[:].rearrange("p (q h) -> p q h", q=4),
                        axis=AX.X)
                    mml = smp.tile([JT, 4], dt, tag="mml")
                    nc.any.tensor_scalar_mul(mml[:], sxn[:],
                                                float(mw[0]) / C)
                    T1v = smp.tile([JT, 4], dt, tag="T1v")
                    nc.vector.scalar_tensor_tensor(
                        out=T1v[:], in0=mml[:],
                        scalar=float(-vw[0] * C / (C - 1.0)
                                     / (mw[0] * mw[0])),
                        in1=mml[:], op0=AO.mult, op1=AO.mult)
                    svla = smp.tile([JT, 4], dt, tag="svla")
                    nc.any.tensor_scalar(
                        out=svla[:], in0=su[:],
                        scalar1=float(vw[0] / (C - 1.0)), scalar2=EPS,
                        op0=AO.mult, op1=AO.add)
                    svl = smp.tile([JT, 4], dt, tag="svl")
                    nc.any.tensor_add(svl[:], svla[:], T1v[:])
                    cur["mmls"].append(mml)
                    cur["svls"].append(svl)

            # -- prev back-half part 3: iv (ACT Rsqrt x8)
            if prev is not None:
                for jc in range(2):
                    V0f = prev[f"V0{jc}"][:].rearrange("p h c -> p (h c)")
                    iv = nmp.tile([JT, 4, C], bt, tag="iv")
                    for q in range(4):
                        _raw_act(nc.scalar, iv[:, q, :], V0f, AF.Rsqrt,
                                 mybir, bias=prev["svls"][jc][:, q:q + 1])
                    prev[f"iv{jc}"] = iv

            if cur is not None:
                # front 4: A-side matmuls (need only xps), then xpq (ACT
                # Square), then B-side matmuls.
                for jc in range(2):
                    pt = ppp.tile([JT, HEADS, 512], dt, tag="pt")
                    cur["pts"].append(pt)
                for jc in range(2):
                    pt = cur["pts"][jc]
                    for h in range(HEADS):
                        hs = slice(h * CH, (h + 1) * CH)
                        for ic in range(2):
                            nc.tensor.matmul(pt[:, h, 0:CH],
                                             pA_sb[ic][:, h, jc, :],
                                             cur["xpss"][ic][:, hs],
                                             start=(ic == 0), stop=(ic == 1))
                for jc in range(2):
                    pt = cur["pts"][jc]
                    for h in range(HEADS):
                        hs = slice(h * CH, (h + 1) * CH)
                        for ic in range(2):
                            nc.tensor.matmul(pt[:, h, CH:2 * CH],
                                             pB_sb[ic][:, h, jc, :],
                                             cur["xpqs"][ic][:, hs],
                                             start=(ic == 0), stop=(ic == 1))

            # -- prev back-half part 4: num + out (DVE 2x) + stores
            if prev is not None:
                pb = prev["b"]
                for jc in range(2):
                    num = nmp.tile([JT, 4, C], bt, tag="num")
                    for q in range(4):
                        nc.vector.tensor_sub(num[:, q], prev["xns"][jc][:, q],
                                             prev[f"Abq{jc}"][:, q])
                    outsb = obp.tile([JT, 4, C], bt, tag="outsb")
                    for q in range(4):
                        nc.vector.tensor_mul(outsb[:, q], num[:, q],
                                             prev[f"iv{jc}"][:, q])
                    osb4 = outsb[:].rearrange("p (d s) c -> p d s c",
                                              d=2, s=2)
                    for d in range(2):
                        for s_ in range(2):
                            nc.gpsimd.dma_start(
                                o_re[pb, jc * 7:(jc + 1) * 7, :, d, s_],
                                osb4[:, d, s_])

            prev = cur

    nc.compile()
    return nc


def _make_in_maps(inputs):
    import ml_dtypes
    x = np.asarray(inputs["x"], dtype=np.float32)
    cs = _host_consts(
        np.asarray(inputs["mean_norm_weight"], dtype=np.float32),
        np.asarray(inputs["var_norm_weight"], dtype=np.float32),
        np.asarray(inputs["pos_w"], dtype=np.float32),
        np.asarray(inputs["pos_b"], dtype=np.float32))
    posA_bf, posB_bf = cs[0], cs[1]
    x_bf = x.astype(ml_dtypes.bfloat16)
    in_maps = []
    for c in range(NCORES):
        m = {"posA": posA_bf, "posB": posB_bf,
             "x": np.ascontiguousarray(
                 x_bf[c * BP:(c + 1) * BP]).reshape(BP, 14, 2, 14, 2, C)}
        in_maps.append(m)
    return in_maps


def kernel(x, weight, bias, mean_norm_weight, var_norm_weight, pos_w, pos_b):
    _ensure_path()
    from concourse import bass_utils

    x = np.asarray(x, dtype=np.float32)
    B = x.shape[0]
    weight = np.asarray(weight, dtype=np.float32)
    bias = np.asarray(bias, dtype=np.float32)

    consts = _host_consts(
        np.asarray(mean_norm_weight, dtype=np.float32),
        np.asarray(var_norm_weight, dtype=np.float32),
        np.asarray(pos_w, dtype=np.float32),
        np.asarray(pos_b, dtype=np.float32))

    key = "v5"
    if key not in _PROGRAM_CACHE:
        _PROGRAM_CACHE[key] = _build_program(consts)
    nc = _PROGRAM_CACHE[key]

    in_maps = _make_in_maps(dict(
        x=x, mean_norm_weight=mean_norm_weight,
        var_norm_weight=var_norm_weight, pos_w=pos_w, pos_b=pos_b))

    res = bass_utils.run_bass_kernel_spmd(nc, in_maps,
                                          core_ids=list(range(NCORES)))
    out = np.concatenate(
        [np.asarray(res.results[c]["out"]).reshape(BP, T, C)
         for c in range(NCORES)], axis=0)
    assert out.shape == (B, T, C)
    out = out.astype(np.float32)

    if np.any(weight != 1.0):
        out = out * weight.reshape(1, 1, C)
    if np.any(bias != 0.0):
        out = out + bias.reshape(1, 1, C)
    return out


# revision 30
# speedup vs baseline: 1.0057x; 1.0057x over previous
"""Trainium2 Bass kernel for nn_DTN_47459388620856 (grouped-moment2 norm +
2x2 pooled positional-attention renormalization).

v5 — bf16 end-to-end, engine rebalance, software pipelining (HW-probed):
  * x cast to bf16 on host; output returned bf16, upcast on host. Halves
    DMA traffic and makes big DVE ops eligible for the 2-byte 2x mode
    (probed: all-bf16 unit-stride tensor_tensor = 2x; broadcast/mixed/
    PSUM = 1x; tensor_scalar with fp32 scalar-AP = 2x; reduce = 1x).
  * DVE (2x): xn via 16 per-(q,h) tensor_scalar_mul, halve-halve+reduce
    for m2/sxn, pooling, per-q num/out, V0 (1x, PSUM read).
  * ACT: xsq Square (split in halves to overlap the DVE m2 chain),
    S rsqrt, xpq/sqa squares, per-q Abq via Identity+bias-AP,
    per-q iv via raw Rsqrt+bias-AP.
  * GpSimd only issues store DMAs: bulk GpSimd work both runs slow
    (~0.5 elem/lane/cycle) and, measured, contends SBUF with DVE,
    slowing DVE 2x ops 2-3x. Keep it idle.
  * sB^2 folded into posB stationary (xpq is a plain square).
  * Emission is software-pipelined: iteration `it` interleaves batch
    `it`'s front half (loads, xsq/m2, S, xn/pool, A-matmuls, xpq,
    B-matmuls) with batch `it-1`'s back half (sqa/V0, Abq, iv, num/out,
    stores) so the in-order DVE/ACT streams rarely stall.
  * Ramp/tail: batch-0 x loads issue before the posA/posB const loads
    (x gates the first xsq; consts aren't needed until the matmuls),
    and stores are split (d,s)-granular for DMA-ring parallelism on
    the final drain.
"""

import numpy as np


def _ensure_path():
    try:
        import concourse  # noqa: F401
    except ImportError:
        import sys
        for p in ("/opt/trn_rl_repo",):
            if p not in sys.path:
                sys.path.insert(0, p)


EPS = 1e-5
HEADS, RES, PS = 4, 28, 14
T, C = RES * RES, 768
CH = C // HEADS
P = PS * PS
JT = 98
NCORES = 8
BP = 4

_PROGRAM_CACHE = {}


def _sigmoid(v):
    return 1.0 / (1.0 + np.exp(-v.astype(np.float64)))


def _host_consts(mean_norm_weight, var_norm_weight, pos_w, pos_b):
    import ml_dtypes
    mw = _sigmoid(mean_norm_weight)
    vw = _sigmoid(var_norm_weight)

    ind = np.arange(PS)[None, :] - np.arange(PS)[:, None]
    indx = np.tile(ind, (PS, PS))
    indy = np.repeat(np.repeat(ind, PS, axis=0), PS, axis=1)
    rel = np.stack([indx, indy, indx * indx + indy * indy], -1).astype(np.float32)
    scores = rel @ pos_w.T.astype(np.float32) + pos_b.astype(np.float32)
    e = np.exp(scores - scores.max(axis=0, keepdims=True))
    pos = e / e.sum(axis=0, keepdims=True)
    pos_h = np.transpose(pos, (2, 0, 1)).astype(np.float64)   # (H, i, j)

    # posA folds (1-mw)/4 so A = posA @ xp_sum = (1-mw)*mean_r;
    # posB folds sB^2 so B = posB @ xp_sum^2 = (1-vw)*mean2_r.
    sA = ((1.0 - mw) / 4.0)
    sB2 = ((1.0 - vw) / 16.0)
    posA = np.zeros((2, JT, HEADS, 2, JT), np.float32)
    posB = np.zeros((2, JT, HEADS, 2, JT), np.float32)
    for ic in range(2):
        for jc in range(2):
            blk = np.transpose(
                pos_h[:, ic * JT:(ic + 1) * JT, jc * JT:(jc + 1) * JT],
                (1, 0, 2))
            posB[ic, :, :, jc, :] = blk * sB2[None, :, None]
            posA[ic, :, :, jc, :] = blk * sA[None, :, None]
    posA_bf = posA.astype(ml_dtypes.bfloat16)
    posB_bf = posB.astype(ml_dtypes.bfloat16)

    sB = (np.sqrt(1.0 - vw) / 4.0).astype(np.float32)
    sG = (np.sqrt(1.0 - vw) / (1.0 - mw)).astype(np.float32)
    return posA_bf, posB_bf, sB, sG, mw.astype(np.float32), vw.astype(np.float32)


def _raw_act(eng, out, in_, func, mybir, bias=0.0, scale=1.0):
    ins = [eng.lower_ap(in_)]
    ins.append(eng.lower_ap(bias) if not isinstance(bias, float)
               else mybir.ImmediateValue(dtype=mybir.dt.float32, value=bias))
    ins.append(mybir.ImmediateValue(dtype=mybir.dt.float32, value=scale))
    ins.append(mybir.ImmediateValue(dtype=mybir.dt.float32, value=0.0))
    return eng.add_instruction(
        mybir.InstActivation(
            name=eng.bass.get_next_instruction_name(),
            func=func, ins=ins, outs=[eng.lower_ap(out)]))


def _build_program(consts):
    _ensure_path()
    from contextlib import ExitStack
    import concourse.bass as bass  # noqa: F401
    import concourse.tile as tile
    from concourse import bacc, mybir

    posA_bf, posB_bf, sB, sG, mw, vw = consts
    eqh = bool(np.all(mw == mw[0]) and np.all(vw == vw[0]))
    assert eqh, "v5 kernel assumes per-head norm weights are equal"

    dt = mybir.dt.float32
    bt = mybir.dt.bfloat16
    AO = mybir.AluOpType
    AF = mybir.ActivationFunctionType
    AX = mybir.AxisListType

    nc = bacc.Bacc("TRN2", target_bir_lowering=False, debug=False,
                   enable_asserts=False)

    x_d = nc.dram_tensor("x", (BP, 14, 2, 14, 2, C), bt,
                         kind="ExternalInput").ap()
    pA_d = nc.dram_tensor("posA", (2, JT, HEADS, 2, JT), bt,
                          kind="ExternalInput").ap()
    pB_d = nc.dram_tensor("posB", (2, JT, HEADS, 2, JT), bt,
                          kind="ExternalInput").ap()
    out_d = nc.dram_tensor("out", (BP, 14, 2, 14, 2, C), bt,
                           kind="ExternalOutput").ap()

    x_re = x_d.transpose([0, 1, 3, 2, 4, 5])
    o_re = out_d.transpose([0, 1, 3, 2, 4, 5])

    with ExitStack() as ctx:
        tc = ctx.enter_context(tile.TileContext(nc))
        cpool = ctx.enter_context(tc.tile_pool(name="consts", bufs=1))
        xtp = ctx.enter_context(tc.tile_pool(name="xt", bufs=3))
        xnp = ctx.enter_context(tc.tile_pool(name="xn", bufs=2))
        xqp = ctx.enter_context(tc.tile_pool(name="xsq", bufs=2))
        smp = ctx.enter_context(tc.tile_pool(name="smalls", bufs=2))
        plp = ctx.enter_context(tc.tile_pool(name="pool", bufs=2))
        abp = ctx.enter_context(tc.tile_pool(name="ab", bufs=2))
        nmp = ctx.enter_context(tc.tile_pool(name="num", bufs=2))
        obp = ctx.enter_context(tc.tile_pool(name="outsb", bufs=2))
        ppp = ctx.enter_context(tc.tile_pool(name="ppsum", bufs=2,
                                             space="PSUM"))

        xt0s = []
        for jc in range(2):
            xt = xtp.tile([JT, 2, 2, C], bt, tag="xt")
            for d in range(2):
                nc.sync.dma_start(xt[:, d],
                                  x_re[0, jc * 7:(jc + 1) * 7, :, d])
            xt0s.append(xt)
        pA_sb, pB_sb = [], []
        for ic in range(2):
            tA = cpool.tile([JT, HEADS, 2, JT], bt, tag=f"posA{ic}")
            nc.sync.dma_start(tA[:], pA_d[ic])
            pA_sb.append(tA)
            tB = cpool.tile([JT, HEADS, 2, JT], bt, tag=f"posB{ic}")
            nc.sync.dma_start(tB[:], pB_d[ic])
            pB_sb.append(tB)

        # Software-pipelined: iteration `it` emits batch `it`'s front half
        # (loads, xsq/m2, S, xn/pool, matmuls) interleaved with batch
        # `it-1`'s back half (sqa/V0, Abq, iv, num/out, stores), so neither
        # the in-order DVE nor the in-order ACT stream ever stalls long.
        prev = None
        for it in range(BP + 1):
            b = it
            # -- prev back-half part 1: sqa (ACT, reads PSUM A-region) + V0
            if prev is not None:
                for jc in range(2):
                    pt = prev["pts"][jc]
                    sqa = abp.tile([JT, HEADS, CH], bt, tag="sqa")
                    nc.scalar.activation(sqa[:], pt[:, :, 0:CH], AF.Square,
                                         scale=float(sG[0]))
                    V0 = abp.tile([JT, HEADS, CH], bt, tag="V0")
                    nc.vector.tensor_sub(V0[:], pt[:, :, CH:2 * CH], sqa[:])
                    prev[f"V0{jc}"] = V0

            cur = None
            if b < BP:
                cur = {"xts": [], "m2s": [], "Ss": [], "xns": [],
                       "xpss": [], "xpqs": [], "mmls": [], "svls": [],
                       "pts": [], "b": b}
                # front 1: loads + xsq + halve + m2 (DVE 2x)
                for jc in range(2):
                    if b == 0:
                        xt = xt0s[jc]
                    else:
                        xt = xtp.tile([JT, 2, 2, C], bt, tag="xt")
                        for d in range(2):
                            nc.sync.dma_start(
                                xt[:, d],
                                x_re[b, jc * 7:(jc + 1) * 7, :, d])
                    cur["xts"].append(xt)
                    xseg = xt[:].rearrange("p d s (h c) -> p (d s h) c",
                                           h=HEADS)
                    xsq = xqp.tile([JT, 16, CH], bt, tag="xsq")
                    m2h = xqp.tile([JT, 16, CH // 2], bt, tag="m2h")
                    m2q = xqp.tile([JT, 16, CH // 4], bt, tag="m2q")
                    m2 = smp.tile([JT, 16], dt, tag="m2")
                    for hf in range(2):
                        sl = slice(hf * 8, (hf + 1) * 8)
                        nc.scalar.activation(xsq[:, sl], xseg[:, sl],
                                             AF.Square)
                        nc.vector.tensor_add(m2h[:, sl],
                                             xsq[:, sl, 0:CH // 2],
                                             xsq[:, sl, CH // 2:CH])
                        nc.vector.tensor_add(m2q[:, sl],
                                             m2h[:, sl, 0:CH // 4],
                                             m2h[:, sl, CH // 4:CH // 2])
                        nc.vector.reduce_sum(m2[:, sl], m2q[:, sl],
                                             axis=AX.X)
                    cur["m2s"].append(m2)
                # front 2: S (ACT Rsqrt)
                for jc in range(2):
                    S = smp.tile([JT, 16], dt, tag="S")
                    _raw_act(nc.scalar, S[:], cur["m2s"][jc][:], AF.Rsqrt,
                             mybir, bias=EPS, scale=1.0 / CH)
                    cur["Ss"].append(S)

            # -- prev back-half part 2: Abq (ACT Identity x8)
            if prev is not None:
                for jc in range(2):
                    pt = prev["pts"][jc]
                    Abq = nmp.tile([JT, 4, C], bt, tag="Abq")
                    for q in range(4):
                        nc.scalar.activation(
                            Abq[:, q, :].rearrange(
                                "p (h c) -> p h c", h=HEADS),
                            pt[:, :, 0:CH], AF.Identity,
                            bias=prev["mmls"][jc][:, q:q + 1])
                    prev[f"Abq{jc}"] = Abq

            if cur is not None:
                # front 3: xn (DVE ts_mul 2x) + sxn + smalls + pooling
                for jc in range(2):
                    xt, m2, S = (cur["xts"][jc], cur["m2s"][jc],
                                 cur["Ss"][jc])
                    xn = xnp.tile([JT, 4, C], bt, tag="xn")
                    for q in range(4):
                        d, s_ = divmod(q, 2)
                        for h in range(HEADS):
                            nc.vector.tensor_scalar_mul(
                                xn[:, q, h * CH:(h + 1) * CH],
                                xt[:, d, s_, h * CH:(h + 1) * CH],
                                S[:, q * HEADS + h:q * HEADS + h + 1])
                    cur["xns"].append(xn)

                    tmp2 = plp.tile([JT, 2, C], bt, tag="tmp2")
                    nc.vector.tensor_add(tmp2[:], xn[:, 0:2, :],
                                         xn[:, 2:4, :])
                    xps = plp.tile([JT, C], bt, tag="xps")
                    nc.vector.tensor_add(xps[:], tmp2[:, 0, :],
                                         tmp2[:, 1, :])
                    cur["xpss"].append(xps)
                    xpq = plp.tile([JT, C], bt, tag="xpq")
                    nc.scalar.activation(xpq[:], xps[:], AF.Square)
                    cur["xpqs"].append(xpq)

                    xnh = xnp.tile([JT, 4, C // 2], bt, tag="xnh")
                    nc.vector.tensor_add(xnh[:], xn[:, :, 0:C // 2],
                                         xn[:, :, C // 2:C])
                    xnq = xnp.tile([JT, 4, C // 4], bt, tag="xnq")
                    nc.vector.tensor_add(xnq[:], xnh[:, :, 0:C // 4],
                                         xnh[:, :, C // 4:C // 2])
                    sxn = smp.tile([JT, 4], dt, tag="sxn")
                    nc.vector.reduce_sum(sxn[:], xnq[:], axis=AX.X)

                    r_ = smp.tile([JT, 16], dt, tag="r")
                    nc.any.tensor_mul(r_[:], S[:], S[:])
                    u = smp.tile([JT, 16], dt, tag="u")
                    nc.any.tensor_mul(u[:], m2[:], r_[:])
                    su = smp.tile([JT, 4], dt, tag="su")
                    nc.vector.reduce_sum(
                        su[:], u[:].rearrange("p (q h) -> p q h", q=4),
                        axis=AX.X)
                    mml = smp.tile([JT, 4], dt, tag="mml")
                    nc.any.tensor_scalar_mul(mml[:], sxn[:],
                                                float(mw[0]) / C)
                    T1v = smp.tile([JT, 4], dt, tag="T1v")
                    nc.vector.scalar_tensor_tensor(
                        out=T1v[:], in0=mml[:],
                        scalar=float(-vw[0] * C / (C - 1.0)
                                     / (mw[0] * mw[0])),
                        in1=mml[:], op0=AO.mult, op1=AO.mult)
                    svla = smp.tile([JT, 4], dt, tag="svla")
                    nc.any.tensor_scalar(
                        out=svla[:], in0=su[:],
                        scalar1=float(vw[0] / (C - 1.0)), scalar2=EPS,
                        op0=AO.mult, op1=AO.add)
                    svl = smp.tile([JT, 4], dt, tag="svl")
                    nc.any.tensor_add(svl[:], svla[:], T1v[:])
                    cur["mmls"].append(mml)
                    cur["svls"].append(svl)

            # -- prev back-half part 3: iv (ACT Rsqrt x8)
            if prev is not None:
                for jc in range(2):
                    V0f = prev[f"V0{jc}"][:].rearrange("p h c -> p (h c)")
                    iv = nmp.tile([JT, 4, C], bt, tag="iv")
                    for q in range(4):
                        _raw_act(nc.scalar, iv[:, q, :], V0f, AF.Rsqrt,
                                 mybir, bias=prev["svls"][jc][:, q:q + 1])
                    prev[f"iv{jc}"] = iv

            if cur is not None:
                # front 4: A-side matmuls (need only xps), then xpq (ACT
                # Square), then B-side matmuls.
                for jc in range(2):
                    pt = ppp.tile([JT, HEADS, 512], dt, tag="pt")
                    cur["pts"].append(pt)
                for jc in range(2):
                    pt = cur["pts"][jc]
                    for h in range(HEADS):
                        hs = slice(h * CH, (h + 1) * CH)
                        for ic in range(2):
                            nc.tensor.matmul(pt[:, h, 0:CH],
                                             pA_sb[ic][:, h, jc, :],
                                             cur["xpss"][ic][:, hs],
                                             start=(ic == 0), stop=(ic == 1))
                for jc in range(2):
                    pt = cur["pts"][jc]
                    for h in range(HEADS):
                        hs = slice(h * CH, (h + 1) * CH)
                        for ic in range(2):
                            nc.tensor.matmul(pt[:, h, CH:2 * CH],
                                             pB_sb[ic][:, h, jc, :],
                                             cur["xpqs"][ic][:, hs],
                                             start=(ic == 0), stop=(ic == 1))

            # -- prev back-half part 4: num + out (DVE 2x) + stores
            if prev is not None:
                pb = prev["b"]
                for jc in range(2):
                    num = nmp.tile([JT, 4, C], bt, tag="num")
                    for q in range(4):
                        nc.vector.tensor_sub(num[:, q], prev["xns"][jc][:, q],
                                             prev[f"Abq{jc}"][:, q])
                    outsb = obp.tile([JT, 4, C], bt, tag="outsb")
                    for q in range(4):
                        nc.vector.tensor_mul(outsb[:, q], num[:, q],
                                             prev[f"iv{jc}"][:, q])
                    osb4 = outsb[:].rearrange("p (d s) c -> p d s c",
                                              d=2, s=2)
                    for d in range(2):
                        for s_ in range(2):
                            nc.gpsimd.dma_start(
                                o_re[pb, jc * 7:(jc + 1) * 7, :, d, s_],
                                osb4[:, d, s_])

            prev = cur

    nc.compile()
    return nc


def _make_in_maps(inputs):
    import ml_dtypes
    x = np.asarray(inputs["x"], dtype=np.float32)
    cs = _host_consts(
        np.asarray(inputs["mean_norm_weight"], dtype=np.float32),
        np.asarray(inputs["var_norm_weight"], dtype=np.float32),
        np.asarray(inputs["pos_w"], dtype=np.float32),
        np.asarray(inputs["pos_b"], dtype=np.float32))
    posA_bf, posB_bf = cs[0], cs[1]
    x_bf = x.astype(ml_dtypes.bfloat16)
    in_maps = []
    for c in range(NCORES):
        m = {"posA": posA_bf, "posB": posB_bf,
             "x": np.ascontiguousarray(
                 x_bf[c * BP:(c + 1) * BP]).reshape(BP, 14, 2, 14, 2, C)}
        in_maps.append(m)
    return in_maps


def kernel(x, weight, bias, mean_norm_weight, var_norm_weight, pos_w, pos_b):
    _ensure_path()
    from concourse import bass_utils

    x = np.asarray(x, dtype=np.float32)
    B = x.shape[0]
    weight = np.asarray(weight, dtype=np.float32)
    bias = np.asarray(bias, dtype=np.float32)

    consts = _host_consts(
        np.asarray(mean_norm_weight, dtype=np.float32),
        np.asarray(var_norm_weight, dtype=np.float32),
        np.asarray(pos_w, dtype=np.float32),
        np.asarray(pos_b, dtype=np.float32))

    key = "v5"
    if key not in _PROGRAM_CACHE:
        _PROGRAM_CACHE[key] = _build_program(consts)
    nc = _PROGRAM_CACHE[key]

    in_maps = _make_in_maps(dict(
        x=x, mean_norm_weight=mean_norm_weight,
        var_norm_weight=var_norm_weight, pos_w=pos_w, pos_b=pos_b))

    res = bass_utils.run_bass_kernel_spmd(nc, in_maps,
                                          core_ids=list(range(NCORES)))
    out = np.concatenate(
        [np.asarray(res.results[c]["out"]).reshape(BP, T, C)
         for c in range(NCORES)], axis=0)
    assert out.shape == (B, T, C)
    out = out.astype(np.float32)

    if np.any(weight != 1.0):
        out = out * weight.reshape(1, 1, C)
    if np.any(bias != 0.0):
        out = out + bias.reshape(1, 1, C)
    return out
